# revision 2
# baseline (speedup 1.0000x reference)
"""ArcMarginProduct + cross-entropy loss, vocab-parallel over 8 NeuronCores.

Math: the reference computes
    cos[b,v] = <x_b/|x_b|, w_v/|w_v|>,  clip to [-1+eps, 1-eps]
    logits   = cos(arccos(cos) + M*onehot(labels))
    loss     = mean(logsumexp(logits, axis=1) - logits[b, label_b])
For v != label_b, cos(arccos(c)) == c, so the only place arccos/cos matter is
the single label column per row -- handled exactly on the host (O(B*D) work).
The device computes, per vocabulary shard, S_partial[b] = sum_v exp(cos[b,v])
(raw, no margin). |cos|<=1 always, so no max-shift is needed for stability.
Host then corrects the label term: S_adj = S - exp(c_label) + exp(c_adj),
loss = mean(log(S_adj) - c_adj).

Sharding: weight columns split V=100000 -> 8 x 12500. Each core gets the full
normalized-transposed input xT[bf16, 512x1024] and its weight shard
w[bf16, 512x12500]. Per 128-class tile: 8 matmuls (classes on PSUM partitions,
batch on the free axis) accumulate over D; column norms come from a
squares-matmul against a ones vector; 1/sqrt(nsq) = Exp(-0.5*Log(nsq)) keeps
ScalarE in one table set; the per-partition scale fuses into the Exp
activation; VectorE accumulates exp tiles into acc[128, B] which the host
reduces.
"""

import sys

if "/opt/trn_rl_repo" not in sys.path:
    sys.path.insert(0, "/opt/trn_rl_repo")

import numpy as np
import ml_dtypes

import concourse.bass as bass
import concourse.mybir as mybir
import concourse.tile as tile
from concourse.bass_utils import run_bass_kernel_spmd

B, D, V = 1024, 512, 100000
NCORES = 8
VS = V // NCORES           # 12500 classes per core
KB = D // 128              # 4 contraction blocks
NVT = (VS + 127) // 128    # 98 class tiles per core (97 full + one of 84)
NB = B // 512              # 2 batch halves (PSUM bank = 512 fp32)
MARGIN = 0.4
EPS = 1e-7

BF16 = mybir.dt.bfloat16
F32 = mybir.dt.float32
AF = mybir.ActivationFunctionType

_nc_cache = []


def _split_multi_waits(nc):
    """This toolchain's walrus accepts at most ONE semaphore wait per
    instruction, but TileContext attaches one wait per producing processor.
    Rewrite any instruction carrying N>1 waits into N-1 same-engine NoOps
    (one wait each) inserted immediately before it; same-engine program order
    keeps the semantics identical."""
    uid = 0
    for f in nc.m.functions:
        for bb in f.blocks:
            insts = bb.instructions
            i = 0
            while i < len(insts):
                inst = insts[i]
                si = inst.sync_info
                if si is not None and len(si.on_wait) > 1:
                    waits = list(si.on_wait)
                    for w in waits[:-1]:
                        uid += 1
                        nop = mybir.InstNoOp(
                            name=f"{inst.name}-wsplit{uid}",
                            engine=inst.engine,
                            sync_info=mybir.SyncInfo(on_wait=[w], on_update=[]),
                            bass_nofuse=True,
                        )
                        insts.insert(i, nop)
                        i += 1
                    inst.sync_info = mybir.SyncInfo(
                        on_wait=[waits[-1]], on_update=list(si.on_update)
                    )
                i += 1


def _build_nc():
    nc = bass.Bass(target_bir_lowering=False)
    xT = nc.declare_dram_parameter("xT", [D, B], BF16, isOutput=False)
    w = nc.declare_dram_parameter("w", [D, VS], BF16, isOutput=False)
    acc_out = nc.declare_dram_parameter("acc", [128, B], F32, isOutput=True)

    xT_r = xT.rearrange("(k p) b -> p k b", p=128)
    w_r = w.rearrange("(k p) v -> p k v", p=128)

    with tile.TileContext(nc) as tc:
        with (
            tc.tile_pool(name="persist", bufs=1) as persist,
            tc.tile_pool(name="wj", bufs=3) as wj_pool,
            tc.tile_pool(name="wsq", bufs=2) as wsq_pool,
            tc.tile_pool(name="small", bufs=4) as small,
            tc.tile_pool(name="expt", bufs=2) as expt_pool,
            tc.tile_pool(name="pm", bufs=2, space="PSUM") as pm_pool,
            tc.tile_pool(name="pn", bufs=2, space="PSUM") as pn_pool,
        ):
            ones = persist.tile([128, 1], BF16, tag="ones")
            nc.vector.memset(ones[:, :], 1.0)
            xt = persist.tile([128, KB, B], BF16, tag="xt")
            nc.sync.dma_start(xt[:, :, :], xT_r[:, :, :])
            acc = persist.tile([128, B], F32, tag="acc")
            nc.vector.memset(acc[:, :], 0.0)

            for j in range(NVT):
                vt = min(128, VS - j * 128)
                wj = wj_pool.tile([128, KB, 128], BF16, tag="wj")
                nc.sync.dma_start(
                    wj[:, :, :vt], w_r[:, :, j * 128 : j * 128 + vt]
                )

                # main matmuls: psum_m[v, b] = sum_d w[d,v] * xT[d,b]
                psum_m = pm_pool.tile([128, B], F32, tag="pm")
                for h in range(NB):
                    for k in range(KB):
                        nc.tensor.matmul(
                            psum_m[:vt, h * 512 : (h + 1) * 512],
                            wj[:, k, :vt],
                            xt[:, k, h * 512 : (h + 1) * 512],
                            start=(k == 0),
                            stop=(k == KB - 1),
                        )

                # column norms: nsq[v] = sum_d w[d,v]^2 via wsq^T @ ones
                wsq = wsq_pool.tile([128, KB, 128], BF16, tag="wsq")
                nc.vector.tensor_mul(
                    wsq[:, :, :vt], wj[:, :, :vt], wj[:, :, :vt]
                )
                psum_n = pn_pool.tile([128, 1], F32, tag="pn")
                for k in range(KB):
                    nc.tensor.matmul(
                        psum_n[:vt, :],
                        wsq[:, k, :vt],
                        ones[:, :],
                        start=(k == 0),
                        stop=(k == KB - 1),
                    )
                # s = 1/sqrt(nsq) = exp(-0.5 * ln(nsq)); ln+exp share one
                # ACT table set (natural_log_exp_and_others)
                ln_n = small.tile([128, 1], F32, tag="ln")
                nc.scalar.activation(ln_n[:vt, :], psum_n[:vt, :], AF.Ln)
                s_col = small.tile([128, 1], F32, tag="scol")
                nc.scalar.activation(
                    s_col[:vt, :], ln_n[:vt, :], AF.Exp, scale=-0.5
                )

                # exp(cos) with the per-class 1/|w_v| fused as ACT scale
                expt = expt_pool.tile([128, B], F32, tag="expt")
                nc.scalar.activation(
                    expt[:vt, :],
                    psum_m[:vt, :],
                    AF.Exp,
                    scale=s_col[:vt, :1],
                )
                nc.vector.tensor_add(
                    acc[:vt, :], acc[:vt, :], expt[:vt, :]
                )

            nc.sync.dma_start(acc_out[:, :], acc[:, :])

    _split_multi_waits(nc)
    return nc


def _get_nc():
    if not _nc_cache:
        _nc_cache.append(_build_nc())
    return _nc_cache[0]


def run_device(in_maps, **kwargs):
    return run_bass_kernel_spmd(_get_nc(), in_maps, list(range(NCORES)), **kwargs)


def make_in_maps(input, weight):
    x = np.asarray(input, dtype=np.float32)
    w = np.asarray(weight, dtype=np.float32)
    x_norm = x / np.maximum(
        np.linalg.norm(x, axis=1, keepdims=True), 1e-12
    )
    xT16 = np.ascontiguousarray(x_norm.T).astype(ml_dtypes.bfloat16)
    w16 = w.astype(ml_dtypes.bfloat16)
    return [
        {"xT": xT16, "w": np.ascontiguousarray(w16[:, i * VS : (i + 1) * VS])}
        for i in range(NCORES)
    ]


def finalize(results, input, weight, labels):
    """Host epilogue: reduce shard partials and apply the exact label-margin
    correction (O(B*D) work)."""
    x = np.asarray(input, dtype=np.float64)
    w = np.asarray(weight, dtype=np.float32)
    lab = np.asarray(labels).astype(np.int64)

    S = np.zeros(B, dtype=np.float64)
    for i in range(NCORES):
        S += results[i]["acc"].astype(np.float64).sum(axis=0)

    x_norm = x / np.maximum(np.linalg.norm(x, axis=1, keepdims=True), 1e-12)
    wl = w[:, lab].astype(np.float64)                    # [D, B]
    wln = np.maximum(np.sqrt((wl * wl).sum(axis=0)), 1e-12)
    c = (x_norm.T * wl).sum(axis=0) / wln                # label cosines
    c = np.clip(c, -1.0 + EPS, 1.0 - EPS)
    c_adj = np.cos(np.arccos(c) + MARGIN)
    S_adj = S - np.exp(c) + np.exp(c_adj)
    logz = np.log(S_adj)
    loss = np.mean(logz - c_adj)
    return np.asarray(loss, dtype=np.float32)


def kernel(input, weight, labels):
    in_maps = make_in_maps(input, weight)
    res = run_device(in_maps)
    return finalize(res.results, input, weight, labels)


# revision 4
# speedup vs baseline: 25.5591x; 25.5591x over previous
"""ArcMarginProduct + cross-entropy loss, vocab-parallel over 8 NeuronCores.

Math: the reference computes
    cos[b,v] = <x_b/|x_b|, w_v/|w_v|>,  clip to [-1+eps, 1-eps]
    logits   = cos(arccos(cos) + M*onehot(labels))
    loss     = mean(logsumexp(logits, axis=1) - logits[b, label_b])
For v != label_b, cos(arccos(c)) == c, so the only place arccos/cos matter is
the single label column per row -- handled exactly on the host (O(B*D) work).
The device computes, per vocabulary shard, S_partial[b] = sum_v exp(cos[b,v])
(raw, no margin). |cos|<=1 always, so no max-shift is needed for stability.
Host then corrects the label term: S_adj = S - exp(c_label) + exp(c_adj),
loss = mean(log(S_adj) - c_adj).

Sharding: weight columns split V=100000 -> 8 x 12500. Each core gets the full
normalized-transposed input xT[bf16, 512x1024] and its weight shard
w[bf16, 512x12500]. Per 128-class tile: 8 matmuls (classes on PSUM partitions,
batch on the free axis) accumulate over D; column norms come from a
squares-matmul against a ones vector; 1/sqrt(nsq) = Exp(-0.5*Ln(nsq)) keeps
ScalarE in one table set; the per-partition scale fuses into the Exp
activation; VectorE accumulates exp tiles into acc[128, B] which the host
reduces.
"""

import contextlib
import sys

if "/opt/trn_rl_repo" not in sys.path:
    sys.path.insert(0, "/opt/trn_rl_repo")

import numpy as np
import ml_dtypes

import concourse.bass as bass
import concourse.mybir as mybir
import concourse.tile as tile
from concourse.bass_utils import run_bass_kernel_spmd

B, D, V = 1024, 512, 100000
NCORES = 8
VS = V // NCORES           # 12500 classes per core
KB = D // 128              # 4 contraction blocks
NVT = (VS + 127) // 128    # 98 class tiles per core (97 full + one of 84)
NB = B // 512              # 2 batch halves (PSUM bank = 512 fp32)
MARGIN = 0.4
EPS = 1e-7

BF16 = mybir.dt.bfloat16
F32 = mybir.dt.float32
AF = mybir.ActivationFunctionType

_nc_cache = {}


def _split_multi_waits(nc):
    """This toolchain's walrus accepts at most ONE semaphore wait per
    instruction, but TileContext attaches one wait per producing processor.
    Rewrite any instruction carrying N>1 waits into N-1 same-engine NoOps
    (one wait each) inserted immediately before it; same-engine program order
    keeps the semantics identical."""
    uid = 0
    for f in nc.m.functions:
        for bb in f.blocks:
            insts = bb.instructions
            i = 0
            while i < len(insts):
                inst = insts[i]
                si = inst.sync_info
                if si is not None and len(si.on_wait) > 1:
                    waits = list(si.on_wait)
                    for w in waits[:-1]:
                        uid += 1
                        nop = mybir.InstNoOp(
                            name=f"{inst.name}-wsplit{uid}",
                            engine=inst.engine,
                            sync_info=mybir.SyncInfo(on_wait=[w], on_update=[]),
                            bass_nofuse=True,
                        )
                        insts.insert(i, nop)
                        i += 1
                    inst.sync_info = mybir.SyncInfo(
                        on_wait=[waits[-1]], on_update=list(si.on_update)
                    )
                i += 1


def _build_nc(repeat=None):
    """repeat=K wraps the workload in a Tile For_i executed K times -- used
    only for timing (amortizes the ~80 ms axon dispatch round-trip); the
    production kernel uses repeat=None."""
    nc = bass.Bass(target_bir_lowering=False)
    xT = nc.declare_dram_parameter("xT", [D, B], BF16, isOutput=False)
    w = nc.declare_dram_parameter("w", [D, VS], BF16, isOutput=False)
    acc_out = nc.declare_dram_parameter("acc", [128, B], F32, isOutput=True)

    xT_r = xT.rearrange("(k p) b -> p k b", p=128)
    w_r = w.rearrange("(k p) v -> p k v", p=128)

    with tile.TileContext(nc) as tc:
        with (
            tc.tile_pool(name="persist", bufs=1) as persist,
            tc.tile_pool(name="wj", bufs=3) as wj_pool,
            tc.tile_pool(name="wsq", bufs=2) as wsq_pool,
            tc.tile_pool(name="small", bufs=4) as small,
            tc.tile_pool(name="expt", bufs=2) as expt_pool,
            tc.tile_pool(name="pm", bufs=2, space="PSUM") as pm_pool,
            tc.tile_pool(name="pn", bufs=2, space="PSUM") as pn_pool,
        ):
            loop_cm = tc.For_i(0, repeat, 1) if repeat else contextlib.nullcontext()
            with loop_cm:
                ones = persist.tile([128, 1], BF16, tag="ones")
                nc.vector.memset(ones[:, :], 1.0)
                xt = persist.tile([128, KB, B], BF16, tag="xt")
                nc.sync.dma_start(xt[:, :, :], xT_r[:, :, :])
                acc = persist.tile([128, B], F32, tag="acc")
                nc.vector.memset(acc[:, :], 0.0)

                for j in range(NVT):
                    vt = min(128, VS - j * 128)
                    wj = wj_pool.tile([128, KB, 128], BF16, tag="wj")
                    nc.sync.dma_start(
                        wj[:, :, :vt], w_r[:, :, j * 128 : j * 128 + vt]
                    )

                    # main matmuls: psum_m[v, b] = sum_d w[d,v] * xT[d,b]
                    psum_m = pm_pool.tile([128, B], F32, tag="pm")
                    for h in range(NB):
                        for k in range(KB):
                            nc.tensor.matmul(
                                psum_m[:vt, h * 512 : (h + 1) * 512],
                                wj[:, k, :vt],
                                xt[:, k, h * 512 : (h + 1) * 512],
                                start=(k == 0),
                                stop=(k == KB - 1),
                            )

                    # column norms: nsq[v] = sum_d w[d,v]^2 via wsq^T @ ones
                    wsq = wsq_pool.tile([128, KB, 128], BF16, tag="wsq")
                    nc.vector.tensor_mul(
                        wsq[:, :, :vt], wj[:, :, :vt], wj[:, :, :vt]
                    )
                    psum_n = pn_pool.tile([128, 1], F32, tag="pn")
                    for k in range(KB):
                        nc.tensor.matmul(
                            psum_n[:vt, :],
                            wsq[:, k, :vt],
                            ones[:, :],
                            start=(k == 0),
                            stop=(k == KB - 1),
                        )
                    # s = 1/sqrt(nsq) = exp(-0.5*ln(nsq)); Ln+Exp share one
                    # ACT table set (natural_log_exp_and_others)
                    ln_n = small.tile([128, 1], F32, tag="ln")
                    nc.scalar.activation(ln_n[:vt, :], psum_n[:vt, :], AF.Ln)
                    s_col = small.tile([128, 1], F32, tag="scol")
                    nc.scalar.activation(
                        s_col[:vt, :], ln_n[:vt, :], AF.Exp, scale=-0.5
                    )

                    # exp(cos) with the per-class 1/|w_v| fused as ACT scale
                    expt = expt_pool.tile([128, B], F32, tag="expt")
                    nc.scalar.activation(
                        expt[:vt, :],
                        psum_m[:vt, :],
                        AF.Exp,
                        scale=s_col[:vt, :1],
                    )
                    nc.vector.tensor_add(
                        acc[:vt, :], acc[:vt, :], expt[:vt, :]
                    )

                nc.sync.dma_start(acc_out[:, :], acc[:, :])

    _split_multi_waits(nc)
    return nc


def _get_nc(repeat=None):
    if repeat not in _nc_cache:
        _nc_cache[repeat] = _build_nc(repeat)
    return _nc_cache[repeat]


def run_device(in_maps, **kwargs):
    return run_bass_kernel_spmd(_get_nc(), in_maps, list(range(NCORES)), **kwargs)


def make_in_maps(input, weight):
    x = np.asarray(input, dtype=np.float32)
    w = np.asarray(weight, dtype=np.float32)
    x_norm = x / np.maximum(
        np.linalg.norm(x, axis=1, keepdims=True), 1e-12
    )
    xT16 = np.ascontiguousarray(x_norm.T).astype(ml_dtypes.bfloat16)
    w16 = w.astype(ml_dtypes.bfloat16)
    return [
        {"xT": xT16, "w": np.ascontiguousarray(w16[:, i * VS : (i + 1) * VS])}
        for i in range(NCORES)
    ]


def finalize(results, input, weight, labels):
    """Host epilogue: reduce shard partials and apply the exact label-margin
    correction (O(B*D) work)."""
    x = np.asarray(input, dtype=np.float64)
    w = np.asarray(weight, dtype=np.float32)
    lab = np.asarray(labels).astype(np.int64)

    S = np.zeros(B, dtype=np.float64)
    for i in range(NCORES):
        S += results[i]["acc"].astype(np.float64).sum(axis=0)

    x_norm = x / np.maximum(np.linalg.norm(x, axis=1, keepdims=True), 1e-12)
    wl = w[:, lab].astype(np.float64)                    # [D, B]
    wln = np.maximum(np.sqrt((wl * wl).sum(axis=0)), 1e-12)
    c = (x_norm.T * wl).sum(axis=0) / wln                # label cosines
    c = np.clip(c, -1.0 + EPS, 1.0 - EPS)
    c_adj = np.cos(np.arccos(c) + MARGIN)
    S_adj = S - np.exp(c) + np.exp(c_adj)
    logz = np.log(S_adj)
    loss = np.mean(logz - c_adj)
    return np.asarray(loss, dtype=np.float32)


def kernel(input, weight, labels):
    in_maps = make_in_maps(input, weight)
    res = run_device(in_maps)
    return finalize(res.results, input, weight, labels)


# revision 7
# speedup vs baseline: 25.5830x; 1.0009x over previous
"""ArcMarginProduct + cross-entropy loss, vocab-parallel over 8 NeuronCores.

Math: the reference computes
    cos[b,v] = <x_b/|x_b|, w_v/|w_v|>,  clip to [-1+eps, 1-eps]
    logits   = cos(arccos(cos) + M*onehot(labels))
    loss     = mean(logsumexp(logits, axis=1) - logits[b, label_b])
For v != label_b, cos(arccos(c)) == c, so the only place arccos/cos matter is
the single label column per row -- handled exactly on the host (O(B*D) work).
The device computes, per vocabulary shard, S_partial[b] = sum_v exp(cos[b,v])
(raw, no margin). |cos|<=1 always, so no max-shift is needed for stability.
Host then corrects the label term: S_adj = S - exp(c_label) + exp(c_adj),
loss = mean(log(S_adj) - c_adj).

Sharding: weight columns split V=100000 -> 8 x 12500. Each core gets the full
normalized-transposed input xT[512x1024] and its weight shard w[512x12500].
Per 128-class tile: classes sit on PSUM partitions, batch on the free axis;
matmuls accumulate over D; column norms come from a squares-matmul against a
ones vector; 1/sqrt(nsq) = Exp(-0.5*Ln(nsq)) keeps ScalarE in one table set
(natural_log_exp_and_others); the per-class 1/|w_v| scale fuses into the Exp
activation as its per-partition scale; VectorE accumulates exp tiles into
acc[128, B] which the host reduces.

Precision: fp8e4m3 operands (host-scaled: x_norm*32, w*256; the 1/32 folds
into the Exp bias, the w-scale cancels inside the self-consistent
normalization) with DoubleRow matmuls (2 fp8 weights per PE cell -> 256-deep
contraction per instruction), fp32 PSUM accumulation, bf16 exp/accumulator.
Verified against the fp32 reference: ~4e-5 relative error on the loss.
"""

import contextlib
import sys

if "/opt/trn_rl_repo" not in sys.path:
    sys.path.insert(0, "/opt/trn_rl_repo")

import math

import numpy as np
import ml_dtypes

import concourse.bass as bass
import concourse.mybir as mybir
import concourse.tile as tile
from concourse.bass_utils import run_bass_kernel_spmd

B, D, V = 1024, 512, 100000
NCORES = 8
VS = V // NCORES           # 12500 classes per core
KB = D // 128              # 4 contraction blocks
NVT = (VS + 127) // 128    # 98 class tiles per core (97 full + one of 84)
NB = B // 512              # 2 batch halves (PSUM bank = 512 fp32)
MARGIN = 0.4
EPS = 1e-7
SX = 32.0                  # fp8 scale for x_norm
SW = 256.0                 # fp8 scale for w (cancels in normalization)

BF16 = mybir.dt.bfloat16
FP8 = mybir.dt.float8e4
F32 = mybir.dt.float32
AF = mybir.ActivationFunctionType
DR = mybir.MatmulPerfMode.DoubleRow

PRECISION = "fp8"          # "fp8" | "bf16"

_nc_cache = {}


def _split_multi_waits(nc):
    """This toolchain's walrus accepts at most ONE semaphore wait per
    instruction, but TileContext attaches one wait per producing processor.
    Rewrite any instruction carrying N>1 waits into N-1 same-engine NoOps
    (one wait each) inserted immediately before it; same-engine program order
    keeps the semantics identical."""
    uid = 0
    for f in nc.m.functions:
        for bb in f.blocks:
            insts = bb.instructions
            i = 0
            while i < len(insts):
                inst = insts[i]
                si = inst.sync_info
                if si is not None and len(si.on_wait) > 1:
                    waits = list(si.on_wait)
                    for w in waits[:-1]:
                        uid += 1
                        nop = mybir.InstNoOp(
                            name=f"{inst.name}-wsplit{uid}",
                            engine=inst.engine,
                            sync_info=mybir.SyncInfo(on_wait=[w], on_update=[]),
                            bass_nofuse=True,
                        )
                        insts.insert(i, nop)
                        i += 1
                    inst.sync_info = mybir.SyncInfo(
                        on_wait=[waits[-1]], on_update=list(si.on_update)
                    )
                i += 1


def _build_nc(repeat=None, precision=None):
    """repeat=K wraps the workload in a Tile For_i executed K times -- used
    only for timing (amortizes the ~80 ms axon dispatch round-trip); the
    production kernel uses repeat=None."""
    precision = precision or PRECISION
    fp8 = precision == "fp8"
    in_dt = FP8 if fp8 else BF16
    acc_dt = BF16 if fp8 else F32

    nc = bass.Bass(target_bir_lowering=False)
    xT = nc.declare_dram_parameter("xT", [D, B], in_dt, isOutput=False)
    w = nc.declare_dram_parameter("w", [D, VS], in_dt, isOutput=False)
    acc_out = nc.declare_dram_parameter("acc", [128, B], acc_dt, isOutput=True)

    xT_r = xT.rearrange("(k p) b -> p k b", p=128)
    w_r = w.rearrange("(k p) v -> p k v", p=128)

    with tile.TileContext(nc) as tc:
        with (
            tc.tile_pool(name="persist", bufs=1) as persist,
            tc.tile_pool(name="wj", bufs=3) as wj_pool,
            tc.tile_pool(name="wsq", bufs=2) as wsq_pool,
            tc.tile_pool(name="small", bufs=4) as small,
            tc.tile_pool(name="expt", bufs=2) as expt_pool,
            tc.tile_pool(name="pm", bufs=2, space="PSUM") as pm_pool,
            tc.tile_pool(name="pn", bufs=2, space="PSUM") as pn_pool,
        ):
            loop_cm = tc.For_i(0, repeat, 1) if repeat else contextlib.nullcontext()
            with loop_cm:
                ones = persist.tile([128, 1], BF16, tag="ones")
                nc.vector.memset(ones[:, :], 1.0)
                bias_col = persist.tile([128, 1], F32, tag="bias")
                nc.vector.memset(bias_col[:, :], -math.log(SX) if fp8 else 0.0)
                xt = persist.tile([128, KB, B], in_dt, tag="xt")
                nc.sync.dma_start(xt[:, :, :], xT_r[:, :, :])
                acc = persist.tile([128, B], acc_dt, tag="acc")
                nc.vector.memset(acc[:, :], 0.0)

                for j in range(NVT):
                    vt = min(128, VS - j * 128)
                    wj = wj_pool.tile([128, KB, 128], in_dt, tag="wj")
                    nc.sync.dma_start(
                        wj[:, :, :vt], w_r[:, :, j * 128 : j * 128 + vt]
                    )

                    # main matmuls: psum_m[v, b] = sum_d w[d,v] * xT[d,b]
                    psum_m = pm_pool.tile([128, B], F32, tag="pm")
                    for h in range(NB):
                        hs = slice(h * 512, (h + 1) * 512)
                        if fp8:
                            # DoubleRow: [128, 2, *] APs contract 256 rows/MM
                            for g in range(KB // 2):
                                nc.tensor.matmul(
                                    psum_m[:vt, hs],
                                    wj[:, 2 * g : 2 * g + 2, :vt],
                                    xt[:, 2 * g : 2 * g + 2, hs],
                                    start=(g == 0),
                                    stop=(g == KB // 2 - 1),
                                    perf_mode=DR,
                                )
                        else:
                            for k in range(KB):
                                nc.tensor.matmul(
                                    psum_m[:vt, hs],
                                    wj[:, k, :vt],
                                    xt[:, k, hs],
                                    start=(k == 0),
                                    stop=(k == KB - 1),
                                )

                    # column norms: nsq[v] = sum_d w[d,v]^2 via wsq^T @ ones
                    wsq = wsq_pool.tile([128, KB, 128], BF16, tag="wsq")
                    nc.vector.tensor_mul(
                        wsq[:, :, :vt], wj[:, :, :vt], wj[:, :, :vt]
                    )
                    psum_n = pn_pool.tile([128, 1], F32, tag="pn")
                    for k in range(KB):
                        nc.tensor.matmul(
                            psum_n[:vt, :],
                            wsq[:, k, :vt],
                            ones[:, :],
                            start=(k == 0),
                            stop=(k == KB - 1),
                        )
                    # s = exp(-0.5*ln(nsq) - ln(SX)) = 1/(SX*sqrt(nsq));
                    # Ln+Exp share one ACT table set
                    ln_n = small.tile([128, 1], F32, tag="ln")
                    nc.scalar.activation(ln_n[:vt, :], psum_n[:vt, :], AF.Ln)
                    s_col = small.tile([128, 1], F32, tag="scol")
                    nc.scalar.activation(
                        s_col[:vt, :],
                        ln_n[:vt, :],
                        AF.Exp,
                        scale=-0.5,
                        bias=bias_col[:vt, :1],
                    )

                    # exp(cos) with the per-class scale fused as ACT scale
                    expt = expt_pool.tile([128, B], acc_dt, tag="expt")
                    nc.scalar.activation(
                        expt[:vt, :],
                        psum_m[:vt, :],
                        AF.Exp,
                        scale=s_col[:vt, :1],
                    )
                    nc.vector.tensor_add(
                        acc[:vt, :], acc[:vt, :], expt[:vt, :]
                    )

                nc.sync.dma_start(acc_out[:, :], acc[:, :])

    _split_multi_waits(nc)
    return nc


def _get_nc(repeat=None, precision=None):
    key = (repeat, precision or PRECISION)
    if key not in _nc_cache:
        _nc_cache[key] = _build_nc(repeat, precision)
    return _nc_cache[key]


def run_device(in_maps, **kwargs):
    return run_bass_kernel_spmd(_get_nc(), in_maps, list(range(NCORES)), **kwargs)


def make_in_maps(input, weight, precision=None):
    precision = precision or PRECISION
    x = np.asarray(input, dtype=np.float32)
    w = np.asarray(weight, dtype=np.float32)
    x_norm = x / np.maximum(
        np.linalg.norm(x, axis=1, keepdims=True), 1e-12
    )
    if precision == "fp8":
        np_dt = ml_dtypes.float8_e4m3
        xT16 = np.ascontiguousarray(x_norm.T * np.float32(SX)).astype(np_dt)
        w16 = (w * np.float32(SW)).astype(np_dt)
    else:
        xT16 = np.ascontiguousarray(x_norm.T).astype(ml_dtypes.bfloat16)
        w16 = w.astype(ml_dtypes.bfloat16)
    return [
        {"xT": xT16, "w": np.ascontiguousarray(w16[:, i * VS : (i + 1) * VS])}
        for i in range(NCORES)
    ]


def finalize(results, input, weight, labels):
    """Host epilogue: reduce shard partials and apply the exact label-margin
    correction (O(B*D) work)."""
    x = np.asarray(input, dtype=np.float64)
    w = np.asarray(weight, dtype=np.float32)
    lab = np.asarray(labels).astype(np.int64)

    S = np.zeros(B, dtype=np.float64)
    for i in range(NCORES):
        S += results[i]["acc"].astype(np.float64).sum(axis=0)

    x_norm = x / np.maximum(np.linalg.norm(x, axis=1, keepdims=True), 1e-12)
    wl = w[:, lab].astype(np.float64)                    # [D, B]
    wln = np.maximum(np.sqrt((wl * wl).sum(axis=0)), 1e-12)
    c = (x_norm.T * wl).sum(axis=0) / wln                # label cosines
    c = np.clip(c, -1.0 + EPS, 1.0 - EPS)
    c_adj = np.cos(np.arccos(c) + MARGIN)
    S_adj = S - np.exp(c) + np.exp(c_adj)
    logz = np.log(S_adj)
    loss = np.mean(logz - c_adj)
    return np.asarray(loss, dtype=np.float32)


def kernel(input, weight, labels):
    in_maps = make_in_maps(input, weight)
    res = run_device(in_maps)
    return finalize(res.results, input, weight, labels)


# revision 11
# speedup vs baseline: 35.4187x; 1.3845x over previous
"""ArcMarginProduct + cross-entropy loss, vocab-parallel over 8 NeuronCores.

Math: the reference computes
    cos[b,v] = <x_b/|x_b|, w_v/|w_v|>,  clip to [-1+eps, 1-eps]
    logits   = cos(arccos(cos) + M*onehot(labels))
    loss     = mean(logsumexp(logits, axis=1) - logits[b, label_b])
For v != label_b, cos(arccos(c)) == c, so the only place arccos/cos matter is
the single label column per row -- handled exactly on the host (O(B*D) work).
The device computes, per vocabulary shard, S_partial[b] = sum_v exp(cos[b,v])
(raw, no margin). |cos|<=1 always, so no max-shift is needed for stability.
Host then corrects the label term: S_adj = S - exp(c_label) + exp(c_adj),
loss = mean(log(S_adj) - c_adj).

Sharding: weight columns split V=100000 -> 8 x 12500. Each core gets the full
normalized-transposed input xT[512x1024] and its weight shard w[512x12500].
Per 128-class tile: classes sit on PSUM partitions, batch on the free axis;
matmuls accumulate over D; column norms come from a squares-matmul against a
ones vector; 1/sqrt(nsq) = Exp(-0.5*Ln(nsq)) keeps ScalarE in one table set
(natural_log_exp_and_others); the per-class 1/|w_v| scale fuses into the Exp
activation as its per-partition scale; VectorE accumulates exp tiles into
acc[128, B] which the host reduces.

Structure: two-phase pipeline interleaved in chunks. Phase 1 loads the
weight shard into SBUF (resident), squares it (VectorE) and reduces the
squares on the PE into per-chunk PSUM columns; one batched Ln+Exp per chunk
produces the fused exp scales. Phase 2 streams the resident weights through
the matmuls, applies Exp with the per-class scale, and accumulates on
VectorE. Chunks interleave (p1c0 p1c1 p2c0 p1c2 p2c1 ...) so DMA/VectorE of
one chunk overlaps PE/ScalarE of the previous.

Precision: fp8e4m3 operands (host-scaled: x_norm*32, w*256; the 1/32 folds
into the Exp bias, the w-scale cancels inside the self-consistent
normalization) with DoubleRow matmuls (2 fp8 weights per PE cell -> 256-deep
contraction per instruction), fp32 PSUM accumulation, bf16 exp/accumulator.
Verified against the fp32 reference: ~4e-5 relative error on the loss.
"""

import contextlib
import math
import sys

if "/opt/trn_rl_repo" not in sys.path:
    sys.path.insert(0, "/opt/trn_rl_repo")

import numpy as np
import ml_dtypes

import concourse.bass as bass
import concourse.mybir as mybir
import concourse.tile as tile
from concourse.bass_utils import run_bass_kernel_spmd

B, D, V = 1024, 512, 100000
NCORES = 8
VS = V // NCORES           # 12500 classes per core
KB = D // 128              # 4 contraction blocks
NVT = (VS + 127) // 128    # 98 class tiles per core (97 full + one of 84)
NB = B // 512              # 2 batch halves (PSUM bank = 512 fp32)
NCHUNK = 4                 # phase-1/phase-2 software pipeline chunks
MARGIN = 0.4
EPS = 1e-7
SX = 32.0                  # fp8 scale for x_norm
SW = 256.0                 # fp8 scale for w (cancels in normalization)

BF16 = mybir.dt.bfloat16
FP8 = mybir.dt.float8e4
F32 = mybir.dt.float32
AF = mybir.ActivationFunctionType
DR = mybir.MatmulPerfMode.DoubleRow

PRECISION = "fp8"          # "fp8" | "bf16"

_nc_cache = {}


def _split_multi_waits(nc):
    """This toolchain's walrus accepts at most ONE semaphore wait per
    instruction, but TileContext attaches one wait per producing processor.
    Rewrite any instruction carrying N>1 waits into N-1 same-engine NoOps
    (one wait each) inserted immediately before it; same-engine program order
    keeps the semantics identical."""
    uid = 0
    for f in nc.m.functions:
        for bb in f.blocks:
            insts = bb.instructions
            i = 0
            while i < len(insts):
                inst = insts[i]
                si = inst.sync_info
                if si is not None and len(si.on_wait) > 1:
                    waits = list(si.on_wait)
                    for w in waits[:-1]:
                        uid += 1
                        nop = mybir.InstNoOp(
                            name=f"{inst.name}-wsplit{uid}",
                            engine=inst.engine,
                            sync_info=mybir.SyncInfo(on_wait=[w], on_update=[]),
                            bass_nofuse=True,
                        )
                        insts.insert(i, nop)
                        i += 1
                    inst.sync_info = mybir.SyncInfo(
                        on_wait=[waits[-1]], on_update=list(si.on_update)
                    )
                i += 1


def _build_nc(repeat=None, precision=None):
    """repeat=K wraps the workload in a Tile For_i executed K times -- used
    only for timing (amortizes the ~80 ms axon dispatch round-trip); the
    production kernel uses repeat=None."""
    precision = precision or PRECISION
    fp8 = precision == "fp8"
    in_dt = FP8 if fp8 else BF16
    acc_dt = BF16 if fp8 else F32

    nc = bass.Bass(target_bir_lowering=False)
    xT = nc.declare_dram_parameter("xT", [D, B], in_dt, isOutput=False)
    w = nc.declare_dram_parameter("w", [D, VS], in_dt, isOutput=False)
    acc_out = nc.declare_dram_parameter("acc", [128, B], acc_dt, isOutput=True)

    xT_r = xT.rearrange("(k p) b -> p k b", p=128)
    w_r = w.rearrange("(k p) v -> p k v", p=128)

    bounds = [round(NVT * c / NCHUNK) for c in range(NCHUNK + 1)]
    chunks = [list(range(bounds[c], bounds[c + 1])) for c in range(NCHUNK)]
    max_nch = max(len(js) for js in chunks)

    with tile.TileContext(nc) as tc:
        with (
            tc.tile_pool(name="persist", bufs=1) as persist,
            tc.tile_pool(name="wall", bufs=1) as wall_pool,
            tc.tile_pool(name="wsq", bufs=3) as wsq_pool,
            tc.tile_pool(name="scol", bufs=2) as scol_pool,
            tc.tile_pool(name="expt", bufs=3) as expt_pool,
            tc.tile_pool(name="pm", bufs=2, space="PSUM") as pm_pool,
            tc.tile_pool(name="pn", bufs=2, space="PSUM") as pn_pool,
        ):
            loop_cm = tc.For_i(0, repeat, 1) if repeat else contextlib.nullcontext()
            with loop_cm:
                ones = persist.tile([128, 1], BF16, tag="ones")
                nc.vector.memset(ones[:, :], 1.0)
                bias_col = persist.tile([128, 1], F32, tag="bias")
                nc.vector.memset(bias_col[:, :], -math.log(SX) if fp8 else 0.0)
                xt = persist.tile([128, KB, B], in_dt, tag="xt")
                nc.sync.dma_start(xt[:, :, :], xT_r[:, :, :])
                acc = persist.tile([128, B], acc_dt, tag="acc")
                nc.vector.memset(acc[:, :], 0.0)
                # whole weight shard stays resident in SBUF (fp8: ~6.3 MB)
                wall = persist.tile([128, NVT, KB, 128], in_dt, tag="wall")
                del wall_pool

                s_chunk = {}

                def phase1(c):
                    js = chunks[c]
                    nch = len(js)
                    psum_n = pn_pool.tile([128, max_nch], F32, tag="pn")
                    for jj, j in enumerate(js):
                        vt = min(128, VS - j * 128)
                        nc.sync.dma_start(
                            wall[:, j, :, :vt],
                            w_r[:, :, j * 128 : j * 128 + vt],
                        )
                        wsq = wsq_pool.tile([128, KB, 128], BF16, tag="wsq")
                        nc.vector.tensor_mul(
                            wsq[:, :, :vt],
                            wall[:, j, :, :vt],
                            wall[:, j, :, :vt],
                        )
                        for k in range(KB):
                            nc.tensor.matmul(
                                psum_n[:vt, jj : jj + 1],
                                wsq[:, k, :vt],
                                ones[:, :],
                                start=(k == 0),
                                stop=(k == KB - 1),
                            )
                    # s = exp(-0.5*ln(nsq) - ln(SX)) = 1/(SX*sqrt(nsq));
                    # Ln+Exp share one ACT table set, batched per chunk
                    ln_n = scol_pool.tile([128, max_nch], F32, tag="ln")
                    nc.scalar.activation(ln_n[:, :nch], psum_n[:, :nch], AF.Ln)
                    s_all = scol_pool.tile([128, max_nch], F32, tag="scol")
                    nc.scalar.activation(
                        s_all[:, :nch],
                        ln_n[:, :nch],
                        AF.Exp,
                        scale=-0.5,
                        bias=bias_col[:, :1],
                    )
                    s_chunk[c] = s_all

                def phase2(c):
                    js = chunks[c]
                    for jj, j in enumerate(js):
                        vt = min(128, VS - j * 128)
                        psum_m = pm_pool.tile([128, B], F32, tag="pm")
                        if fp8:
                            # DoubleRow: [128, 2, *] APs contract 256 rows
                            for g in range(KB // 2):
                                for h in range(NB):
                                    hs = slice(h * 512, (h + 1) * 512)
                                    nc.tensor.matmul(
                                        psum_m[:vt, hs],
                                        wall[:, j, 2 * g : 2 * g + 2, :vt],
                                        xt[:, 2 * g : 2 * g + 2, hs],
                                        start=(g == 0),
                                        stop=(g == KB // 2 - 1),
                                        perf_mode=DR,
                                    )
                        else:
                            for k in range(KB):
                                for h in range(NB):
                                    hs = slice(h * 512, (h + 1) * 512)
                                    nc.tensor.matmul(
                                        psum_m[:vt, hs],
                                        wall[:, j, k, :vt],
                                        xt[:, k, hs],
                                        start=(k == 0),
                                        stop=(k == KB - 1),
                                    )
                        expt = expt_pool.tile([128, B], acc_dt, tag="expt")
                        nc.scalar.activation(
                            expt[:vt, :],
                            psum_m[:vt, :],
                            AF.Exp,
                            scale=s_chunk[c][:vt, jj : jj + 1],
                        )
                        nc.vector.tensor_add(
                            acc[:vt, :], acc[:vt, :], expt[:vt, :]
                        )

                # software pipeline: p1(c0) p1(c1) p2(c0) p1(c2) p2(c1) ...
                phase1(0)
                for c in range(1, NCHUNK):
                    phase1(c)
                    phase2(c - 1)
                phase2(NCHUNK - 1)

                nc.sync.dma_start(acc_out[:, :], acc[:, :])

    _split_multi_waits(nc)
    return nc


def _get_nc(repeat=None, precision=None):
    key = (repeat, precision or PRECISION)
    if key not in _nc_cache:
        _nc_cache[key] = _build_nc(repeat, precision)
    return _nc_cache[key]


def run_device(in_maps, **kwargs):
    return run_bass_kernel_spmd(_get_nc(), in_maps, list(range(NCORES)), **kwargs)


def make_in_maps(input, weight, precision=None):
    precision = precision or PRECISION
    x = np.asarray(input, dtype=np.float32)
    w = np.asarray(weight, dtype=np.float32)
    x_norm = x / np.maximum(
        np.linalg.norm(x, axis=1, keepdims=True), 1e-12
    )
    if precision == "fp8":
        np_dt = ml_dtypes.float8_e4m3
        xT16 = np.ascontiguousarray(x_norm.T * np.float32(SX)).astype(np_dt)
        w16 = (w * np.float32(SW)).astype(np_dt)
    else:
        xT16 = np.ascontiguousarray(x_norm.T).astype(ml_dtypes.bfloat16)
        w16 = w.astype(ml_dtypes.bfloat16)
    return [
        {"xT": xT16, "w": np.ascontiguousarray(w16[:, i * VS : (i + 1) * VS])}
        for i in range(NCORES)
    ]


def finalize(results, input, weight, labels):
    """Host epilogue: reduce shard partials and apply the exact label-margin
    correction (O(B*D) work)."""
    x = np.asarray(input, dtype=np.float64)
    w = np.asarray(weight, dtype=np.float32)
    lab = np.asarray(labels).astype(np.int64)

    S = np.zeros(B, dtype=np.float64)
    for i in range(NCORES):
        S += results[i]["acc"].astype(np.float64).sum(axis=0)

    x_norm = x / np.maximum(np.linalg.norm(x, axis=1, keepdims=True), 1e-12)
    wl = w[:, lab].astype(np.float64)                    # [D, B]
    wln = np.maximum(np.sqrt((wl * wl).sum(axis=0)), 1e-12)
    c = (x_norm.T * wl).sum(axis=0) / wln                # label cosines
    c = np.clip(c, -1.0 + EPS, 1.0 - EPS)
    c_adj = np.cos(np.arccos(c) + MARGIN)
    S_adj = S - np.exp(c) + np.exp(c_adj)
    logz = np.log(S_adj)
    loss = np.mean(logz - c_adj)
    return np.asarray(loss, dtype=np.float32)


def kernel(input, weight, labels):
    in_maps = make_in_maps(input, weight)
    res = run_device(in_maps)
    return finalize(res.results, input, weight, labels)
